# revision 1
# baseline (speedup 1.0000x reference)
"""Multi-head self-attention TRN2 kernel (data-parallel over batch).

Problem: B=8, S=1024, D=384, H=8, per-head full D->D projections,
causal + key-padding mask, softmax, out_linear (H*D)->D, query-mask output.

Sharding: batch b -> NeuronCore b (8 cores, no collectives).

Per-core dataflow (one batch element), transpose-free "T-native" layout:
  xT [D,S] resident in SBUF.
  For each head h:
    QT[e,s], KT[e,t] via (lhsT=W chunk, rhs=xT) matmuls (+bias)
    V[t,e] natural via (lhsT=xT chunk, rhs=Wv)   (+bias bcast)
    per q-tile group (4 q-tiles = 512 s-columns), causally-live t-chunks only:
      scoresT[t,s] psum = KT-stationary @ QT       (raw, unscaled)
      masked = min(scoresT, MT[t-chunk])           (DVE, in-psum)
      attnT[t,s] = exp(masked * inv_sqrt_d)        (ACT, direct to f32r SBUF)
      colsums[1,s] += ones^T @ attnT               (PE, M=1 matmul)
    colsums -> DRAM bounce -> [128,4] per-qt columns -> reciprocal
    headT[e,s] = V-stationary @ attnT              (PE)
    out_acc[s,:] += (headT^T @ Wo_h) * recip[s]    (PE + one DVE STT)
  out[s,:] = out_acc * maskq[s]  -> DRAM

No row-max subtraction: masked fill NEG scales to exactly -87, so exp args
stay in [-87, ~25], safe in fp32. All-invalid rows would be wrong (uniform
over a partial window) but those rows always have maskq[s]=0 and are zeroed.
Masking uses min(scores, MT) where MT[t,s] = +BIG if (t<=s and mask[t])
else NEG, matching the reference's where(valid, scores, -1e4) semantics.
"""

import os
from contextlib import ExitStack

import numpy as np

B, S, D, H = 8, 1024, 384, 8
P = 128
DC = D // P          # 3 partition chunks of the d/e axes
NQT = S // P         # 8 q/t tiles of 128
G = 4                # q-tiles per group (s-block = 512)
NG = NQT // G
BIG = 3.0e38
INV_SQRT_D = float(1.0 / np.sqrt(np.float32(D), dtype=np.float32))
NEG = float(-87.0 / INV_SQRT_D)  # raw-score fill; scaled -> -87

# matmul dtype knobs per stage: "f32" | "f32r" | "bf16"
CFG = {
    "proj": os.environ.get("MHA_DT_PROJ", "f32r"),
    "qk": os.environ.get("MHA_DT_QK", "f32r"),
    "pv": os.environ.get("MHA_DT_PV", "f32r"),
    "op": os.environ.get("MHA_DT_OP", "f32r"),
}

_BUILT = None  # (nc, cfg)


def _dt(kind):
    import concourse.mybir as mybir

    if kind == "bf16":
        return mybir.dt.bfloat16
    if kind == "f32r":
        return mybir.dt.float32r
    return mybir.dt.float32


def _np_dt(kind):
    import ml_dtypes

    return ml_dtypes.bfloat16 if kind == "bf16" else np.float32


def build(cfg=None):
    import concourse.bass as bass
    import concourse.bacc as bacc
    import concourse.tile as tile
    import concourse.mybir as mybir

    cfg = dict(CFG if cfg is None else cfg)
    f32 = mybir.dt.float32
    u32 = mybir.dt.uint32
    dt_proj = _dt(cfg["proj"])   # xT, Wq, Wk, Wv tiles
    dt_qk = _dt(cfg["qk"])       # QT, KT tiles
    dt_pv = _dt(cfg["pv"])       # attnT, V, ones tiles
    dt_op = _dt(cfg["op"])       # headT, Wo tiles

    nc = bacc.Bacc("TRN2", target_bir_lowering=False, debug=False)

    xT_d = nc.dram_tensor("xT", [D, S], dt_proj, kind="ExternalInput")
    wq_d = nc.dram_tensor("Wq", [H, D, D], dt_proj, kind="ExternalInput")
    wk_d = nc.dram_tensor("Wk", [H, D, D], dt_proj, kind="ExternalInput")
    wv_d = nc.dram_tensor("Wv", [H, D, D], dt_proj, kind="ExternalInput")
    wo_d = nc.dram_tensor("Wo", [H * D, D], dt_op, kind="ExternalInput")
    bq_d = nc.dram_tensor("bq", [H, D], f32, kind="ExternalInput")
    bk_d = nc.dram_tensor("bk", [H, D], f32, kind="ExternalInput")
    bv_d = nc.dram_tensor("bv", [H, P, D], f32, kind="ExternalInput")
    bo_d = nc.dram_tensor("bo", [P, D], f32, kind="ExternalInput")
    kbigT_d = nc.dram_tensor("kbigT", [P, NQT], f32, kind="ExternalInput")
    maskq_d = nc.dram_tensor("maskq", [S], f32, kind="ExternalInput")
    out_d = nc.dram_tensor("out", [S, D], f32, kind="ExternalOutput")
    # per-(head, group) bounce rows for column sums
    scr_d = nc.dram_tensor("sum_scratch", [H * NG, 512], f32)

    with tile.TileContext(nc) as tc, ExitStack() as ctx:
        consts = ctx.enter_context(tc.tile_pool(name="consts", bufs=1))
        wpool = ctx.enter_context(tc.tile_pool(name="wpool", bufs=2))
        qkv = ctx.enter_context(tc.tile_pool(name="qkv", bufs=1))
        tpool = ctx.enter_context(tc.tile_pool(name="tpool", bufs=1))
        hpool = ctx.enter_context(tc.tile_pool(name="hpool", bufs=2))
        small = ctx.enter_context(tc.tile_pool(name="small", bufs=8))
        opool = ctx.enter_context(tc.tile_pool(name="opool", bufs=2))
        ps_a = ctx.enter_context(tc.tile_pool(name="ps_a", bufs=4, space="PSUM"))
        ps_sm = ctx.enter_context(tc.tile_pool(name="ps_sm", bufs=2, space="PSUM"))
        ps_u = ctx.enter_context(tc.tile_pool(name="ps_u", bufs=2, space="PSUM"))

        # ---- PE warm-up: keep the array busy under the initial DMA shadow
        # so the HAM clock-gate is released (2.4 GHz) before real matmuls.
        warm = consts.tile([P, P], dt_proj, tag="warm")
        wz = warm.bitcast(u32) if dt_proj == mybir.dt.float32r else warm
        nc.vector.memset(wz, 0)
        ps_w = ps_sm.tile([P, 512], f32, tag="sm", name="ps_warm")
        for _ in range(24):
            nc.tensor.matmul(ps_w[:, :P], warm, warm, start=True, stop=True)

        # ---- setup ----
        xT_sb = consts.tile([P, DC, S], dt_proj, tag="xT")
        nc.sync.dma_start(out=xT_sb, in_=xT_d.ap().rearrange("(c p) s -> p c s", p=P))

        kbigT_sb = consts.tile([P, NQT], f32, tag="kbigT")
        nc.sync.dma_start(out=kbigT_sb, in_=kbigT_d.ap())

        maskq_sb = consts.tile([P, NQT], f32, tag="maskq")
        nc.sync.dma_start(
            out=maskq_sb, in_=maskq_d.ap().rearrange("(q p) -> p q", p=P)
        )

        bo_sb = consts.tile([P, D], f32, tag="bo")
        nc.sync.dma_start(out=bo_sb, in_=bo_d.ap())

        ones_sb = consts.tile([P, 1], dt_pv, tag="ones")
        if dt_pv == mybir.dt.float32r:
            nc.vector.memset(ones_sb.bitcast(u32), 0x3F800000)
        else:
            nc.vector.memset(ones_sb, 1.0)

        # MT[t, s] = kbig[t] where s >= t else NEG   (per 128-chunk of t)
        msk = consts.tile([P, NQT, S], f32, tag="M")
        for tt in range(NQT):
            nc.vector.memset(msk[:, tt, :], 0.0)
            nc.vector.tensor_scalar_add(
                out=msk[:, tt, :], in0=msk[:, tt, :],
                scalar1=kbigT_sb[:, tt : tt + 1],
            )
            nc.gpsimd.affine_select(
                out=msk[:, tt, :],
                in_=msk[:, tt, :],
                compare_op=mybir.AluOpType.is_ge,
                fill=NEG,
                base=-tt * P,
                channel_multiplier=-1,
                pattern=[[1, S]],
            )

        out_acc = consts.tile([P, NQT, D], f32, tag="out_acc")
        for qt in range(NQT):
            nc.vector.tensor_copy(out=out_acc[:, qt, :], in_=bo_sb)

        # ---- per-head pipeline ----
        n_heads = int(os.environ.get("MHA_HEADS", str(H)))
        for h in range(n_heads):
            wq_sb = wpool.tile([P, DC, D], dt_proj, tag="wq")
            wk_sb = wpool.tile([P, DC, D], dt_proj, tag="wk")
            wv_sb = wpool.tile([P, DC, D], dt_proj, tag="wv")
            wo_sb = wpool.tile([P, DC, D], dt_op, tag="wo")
            nc.sync.dma_start(
                out=wq_sb, in_=wq_d.ap()[h].rearrange("(c p) e -> p c e", p=P)
            )
            nc.sync.dma_start(
                out=wk_sb, in_=wk_d.ap()[h].rearrange("(c p) e -> p c e", p=P)
            )
            nc.sync.dma_start(
                out=wv_sb, in_=wv_d.ap()[h].rearrange("(c p) e -> p c e", p=P)
            )
            nc.sync.dma_start(
                out=wo_sb,
                in_=wo_d.ap()[h * D : (h + 1) * D, :].rearrange(
                    "(c p) e -> p c e", p=P
                ),
            )
            bq_sb = wpool.tile([P, DC], f32, tag="bq")
            bk_sb = wpool.tile([P, DC], f32, tag="bk")
            nc.sync.dma_start(out=bq_sb, in_=bq_d.ap()[h].rearrange("(c p) -> p c", p=P))
            nc.sync.dma_start(out=bk_sb, in_=bk_d.ap()[h].rearrange("(c p) -> p c", p=P))
            bv_sb = wpool.tile([P, D], f32, tag="bv")
            nc.sync.dma_start(out=bv_sb, in_=bv_d.ap()[h])

            # QT/KT [e, s] projections (psum 512-wide per (ec, sh))
            qt_sb = qkv.tile([P, DC, S], dt_qk, tag="QT")
            kt_sb = qkv.tile([P, DC, S], dt_qk, tag="KT")
            for dst, w_sb, b_sb in ((kt_sb, wk_sb, bk_sb), (qt_sb, wq_sb, bq_sb)):
                for ec in range(DC):
                    for sh in range(S // 512):
                        ps = ps_a.tile([P, 512], f32, tag="a")
                        for dc in range(DC):
                            nc.tensor.matmul(
                                ps,
                                w_sb[:, dc, ec * P : (ec + 1) * P],
                                xT_sb[:, dc, sh * 512 : (sh + 1) * 512],
                                start=(dc == 0),
                                stop=(dc == DC - 1),
                            )
                        nc.scalar.activation(
                            out=dst[:, ec, sh * 512 : (sh + 1) * 512],
                            in_=ps,
                            func=mybir.ActivationFunctionType.Identity,
                            bias=b_sb[:, ec : ec + 1],
                        )

            # V [t, e] natural
            v_sb = qkv.tile([P, NQT, D], dt_pv, tag="V")
            for tt in range(NQT):
                psv = ps_sm.tile([P, 512], f32, tag="sm")
                for dc in range(DC):
                    nc.tensor.matmul(
                        psv[:, :D],
                        xT_sb[:, dc, tt * P : (tt + 1) * P],
                        wv_sb[:, dc, :],
                        start=(dc == 0),
                        stop=(dc == DC - 1),
                    )
                nc.vector.tensor_add(out=v_sb[:, tt, :], in0=psv[:, :D], in1=bv_sb)

            # attention per 512-wide s-group, causally-live t-chunks only
            for qg in range(NG):
                ntt = qg * G + G  # live t-chunks for this group
                s0 = qg * 512
                att_t = tpool.tile([P, NQT, 512], dt_pv, tag="attnT", name="att_t")
                ps_sums = ps_u.tile([1, 512], f32, tag="u")
                for tt in range(ntt):
                    ps_sc = ps_a.tile([P, 512], f32, tag="a")
                    for ec in range(DC):
                        nc.tensor.matmul(
                            ps_sc,
                            kt_sb[:, ec, tt * P : (tt + 1) * P],
                            qt_sb[:, ec, s0 : s0 + 512],
                            start=(ec == 0),
                            stop=(ec == DC - 1),
                        )
                    nc.vector.tensor_tensor(
                        out=ps_sc,
                        in0=ps_sc,
                        in1=msk[:, tt, s0 : s0 + 512],
                        op=mybir.AluOpType.min,
                    )
                    nc.scalar.activation(
                        out=att_t[:, tt, :],
                        in_=ps_sc,
                        func=mybir.ActivationFunctionType.Exp,
                        scale=INV_SQRT_D,
                    )
                for tt in range(ntt):
                    nc.tensor.matmul(
                        ps_sums,
                        ones_sb,
                        att_t[:, tt, :],
                        start=(tt == 0),
                        stop=(tt == ntt - 1),
                    )
                # column sums -> DRAM bounce -> per-qt [128,1] recips
                srow = small.tile([1, 512], f32, tag="srow")
                nc.vector.tensor_copy(out=srow, in_=ps_sums)
                scr = scr_d.ap()[h * NG + qg]
                nc.sync.dma_start(out=scr, in_=srow)
                scat = small.tile([P, G], f32, tag="scat")
                nc.sync.dma_start(
                    out=scat,
                    in_=bass.AP(tensor=scr.tensor, offset=scr.offset, ap=[[1, P], [P, G]]),
                )
                recip = small.tile([P, G], f32, tag="recip")
                nc.vector.reciprocal(out=recip, in_=scat)

                # headT [e, s-group] = sum_t V-stationary @ attnT
                head_sb = hpool.tile([P, DC, 512], dt_op, tag="headT")
                for ec in range(DC):
                    pspv = ps_sm.tile([P, 512], f32, tag="sm")
                    for tt in range(ntt):
                        nc.tensor.matmul(
                            pspv,
                            v_sb[:, tt, ec * P : (ec + 1) * P],
                            att_t[:, tt, :],
                            start=(tt == 0),
                            stop=(tt == ntt - 1),
                        )
                    nc.scalar.copy(out=head_sb[:, ec, :], in_=pspv)

                # out projection for this head, accumulate with 1/colsum
                for qi in range(G):
                    qt = qg * G + qi
                    psop = ps_sm.tile([P, 512], f32, tag="sm")
                    for ec in range(DC):
                        nc.tensor.matmul(
                            psop[:, :D],
                            head_sb[:, ec, qi * P : (qi + 1) * P],
                            wo_sb[:, ec, :],
                            start=(ec == 0),
                            stop=(ec == DC - 1),
                        )
                    nc.vector.scalar_tensor_tensor(
                        out=out_acc[:, qt, :],
                        in0=psop[:, :D],
                        scalar=recip[:, qi : qi + 1],
                        in1=out_acc[:, qt, :],
                        op0=mybir.AluOpType.mult,
                        op1=mybir.AluOpType.add,
                    )

        # ---- final query-mask + store ----
        for qt in range(NQT):
            st = opool.tile([P, D], f32, tag="store")
            nc.vector.tensor_scalar_mul(
                out=st, in0=out_acc[:, qt, :], scalar1=maskq_sb[:, qt : qt + 1]
            )
            nc.sync.dma_start(out=out_d.ap()[qt * P : (qt + 1) * P, :], in_=st)

    nc.compile()
    return nc


def _in_maps(x, mask, Wq, bq, Wk, bk, Wv, bv, Wo, bo, cfg):
    np_proj = _np_dt(cfg["proj"])
    np_op = _np_dt(cfg["op"])
    x = np.asarray(x, np.float32)
    xT = np.ascontiguousarray(x.transpose(0, 2, 1))  # [B, D, S]
    m = np.asarray(mask) != 0
    kbig = np.where(m, np.float32(BIG), np.float32(NEG)).astype(np.float32)
    maskq = m.astype(np.float32)
    shared = {
        "Wq": np.asarray(Wq, np_proj),
        "Wk": np.asarray(Wk, np_proj),
        "Wv": np.asarray(Wv, np_proj),
        "Wo": np.asarray(Wo, np_op),
        "bq": np.asarray(bq, np.float32),
        "bk": np.asarray(bk, np.float32),
        "bv": np.broadcast_to(
            np.asarray(bv, np.float32)[:, None, :], (H, P, D)
        ).copy(),
        "bo": np.broadcast_to(np.asarray(bo, np.float32)[None, :], (P, D)).copy(),
    }
    return [
        {
            "xT": xT[b].astype(np_proj),
            "kbigT": np.ascontiguousarray(kbig[b].reshape(NQT, P).T),
            "maskq": maskq[b],
            **shared,
        }
        for b in range(B)
    ]


def run(inputs, trace=False, cfg=None):
    """inputs: dict from setup_inputs(). Returns (out [B,S,D] f32, results)."""
    from concourse.bass_utils import run_bass_kernel_spmd

    global _BUILT
    cfg = dict(CFG if cfg is None else cfg)
    if _BUILT is None or _BUILT[1] != cfg:
        _BUILT = (build(cfg), cfg)
    nc = _BUILT[0]
    in_maps = _in_maps(**inputs, cfg=cfg)
    res = run_bass_kernel_spmd(
        nc, in_maps, core_ids=list(range(B)), trace=trace
    )
    out = np.stack([np.asarray(res.results[b]["out"], np.float32) for b in range(B)])
    return out, res


def kernel(**inputs):
    out, _ = run(inputs, trace=False)
    return out



# revision 2
# speedup vs baseline: 1.7761x; 1.7761x over previous
"""Multi-head self-attention TRN2 kernel (data-parallel over batch).

Problem: B=8, S=1024, D=384, H=8, per-head full D->D projections,
causal + key-padding mask, softmax, out_linear (H*D)->D, query-mask output.

Sharding: batch b -> NeuronCore b (8 cores, no collectives).

Host-side weight folding (exact, fp32 numpy; biases handled exactly):
  A_h = Wq_h @ Wk_h^T  -> scores_raw = x A_h x^T + 1 (x wr)^T  with
        wr = Wk_h bq_h folded as a bias into TA = x A_h (the bk rank-1
        term and bq.bk constant are per-query-row and softmax-invariant).
        The K projection disappears: the QK matmul contracts x directly
        against TA.
  C_h = Wv_h @ Wo_h    -> out = sum_h attn_h (x C_h) + bo2 with
        bo2 = bo + sum_h bv_h Wo_h (attn rows sum to 1). V projection and
        out projection collapse into one x @ C_h, and the PV matmul emits
        output-space columns directly.

Per-core dataflow (one batch element), all matmul operands bf16 (f32 psum):
  xTbf [e,s] bf16 resident in SBUF.
  For each head h:
    TA[e',s] (T-layout) = A chunkT @ xTbf (+wr bias via ACT identity)
    U[t,0:384] = xTbf chunkT @ C; U[t,384] = 1   (fused softmax denominator)
    per 512-wide s-group, causally live t-chunks, diagonal chunks trimmed
    to their live s-suffix (N = 512,384,256,128):
      scoresT[t,s] psum = xTbf chunk (lhsT) @ TA          (PE)
      att[t,s] = exp(scale*scores + logm[t])  bf16        (ACT; logm = -60
                 for mask-dead keys else 0 -- keeps dead-row colsums
                 normal-range so 1/colsum never overflows)
      diagonal chunks: zero the causally-dead triangle    (GPSIMD affine)
    per 128-wide q-tile qt (true causal granularity):
      psum[s, 0:385] = sum_{tt<=qt} att_ttT @ U_tt        (PE, one psum)
      recip = 1/psum[:,384:385]                           (DVE, per-s)
      out_acc[s,:] += psum[:, :384] * recip               (DVE STT)
  out[s,:] = out_acc * maskq[s]  -> DRAM

No DRAM bounce, no separate colsum matmuls, no mask tiles: the softmax
denominator rides along as U's 385th column, and masking is an ACT bias
plus a GPSIMD affine_select. bf16 rounding keeps rel err ~1e-2 max-normed,
inside the 2e-2 gate.
"""

import os
from contextlib import ExitStack

import numpy as np

B, S, D, H = 8, 1024, 384, 8
P = 128
DC = D // P          # 3 partition chunks of the d/e axes
NQT = S // P         # 8 q/t tiles of 128
G4 = 4               # q-tiles per s-group (s-block = 512)
NG = NQT // G4       # 2 groups
D1 = D + 1           # U carries a ones column for the softmax denominator
INV_SQRT_D = float(1.0 / np.sqrt(np.float32(D), dtype=np.float32))
LOGM_DEAD = -60.0    # exp bias for mask-dead keys: e^(score-60) stays normal

_BUILT = None


def build():
    import concourse.bass as bass
    import concourse.bacc as bacc
    import concourse.tile as tile
    import concourse.mybir as mybir

    f32 = mybir.dt.float32
    bf16 = mybir.dt.bfloat16

    nc = bacc.Bacc("TRN2", target_bir_lowering=False, debug=False)

    xT_d = nc.dram_tensor("xT", [D, S], bf16, kind="ExternalInput")
    a_d = nc.dram_tensor("A", [H, D, D], bf16, kind="ExternalInput")
    c_d = nc.dram_tensor("C", [H, D, D], bf16, kind="ExternalInput")
    wr_d = nc.dram_tensor("wr", [H, D], f32, kind="ExternalInput")
    bo2_d = nc.dram_tensor("bo2", [P, D], f32, kind="ExternalInput")
    logm_d = nc.dram_tensor("logmT", [P, NQT], f32, kind="ExternalInput")
    maskq_d = nc.dram_tensor("maskqT", [P, NQT], f32, kind="ExternalInput")
    out_d = nc.dram_tensor("out", [S, D], f32, kind="ExternalOutput")

    with tile.TileContext(nc) as tc, ExitStack() as ctx:
        consts = ctx.enter_context(tc.tile_pool(name="consts", bufs=1))
        wpool = ctx.enter_context(tc.tile_pool(name="wpool", bufs=2))
        tatp = ctx.enter_context(tc.tile_pool(name="tatp", bufs=2))
        upool = ctx.enter_context(tc.tile_pool(name="upool", bufs=2))
        attp = ctx.enter_context(tc.tile_pool(name="attp", bufs=2))
        small = ctx.enter_context(tc.tile_pool(name="small", bufs=8))
        opool = ctx.enter_context(tc.tile_pool(name="opool", bufs=2))
        ps_w = ctx.enter_context(tc.tile_pool(name="ps_w", bufs=3, space="PSUM"))
        ps_qk = ctx.enter_context(tc.tile_pool(name="ps_qk", bufs=3, space="PSUM"))
        ps_pv = ctx.enter_context(tc.tile_pool(name="ps_pv", bufs=2, space="PSUM"))

        # ---- PE warm-up under the initial DMA shadow (HAM clock ramp) ----
        warm = consts.tile([P, P], bf16, tag="warm")
        nc.vector.memset(warm, 0.0)
        wz = consts.tile([P, 512], bf16, tag="warmz")
        nc.vector.memset(wz, 0.0)
        for _ in range(20):
            ps = ps_w.tile([P, 512], f32, tag="w", name="ps_warm")
            nc.tensor.matmul(ps, warm, wz, start=True, stop=True)

        # ---- setup ----
        xT_sb = consts.tile([P, DC, S], bf16, tag="xT")
        nc.sync.dma_start(out=xT_sb, in_=xT_d.ap().rearrange("(c p) s -> p c s", p=P))

        logm_sb = consts.tile([P, NQT], f32, tag="logm")
        nc.sync.dma_start(out=logm_sb, in_=logm_d.ap())

        maskq_sb = consts.tile([P, NQT], f32, tag="maskq")
        nc.sync.dma_start(out=maskq_sb, in_=maskq_d.ap())

        bo2_sb = consts.tile([P, D], f32, tag="bo2")
        nc.sync.dma_start(out=bo2_sb, in_=bo2_d.ap())

        out_acc = consts.tile([P, NQT, D], f32, tag="out_acc")
        for qt in range(NQT):
            nc.vector.tensor_copy(out=out_acc[:, qt, :], in_=bo2_sb)

        # ---- per-head pipeline ----
        n_heads = int(os.environ.get("MHA_HEADS", str(H)))
        for h in range(n_heads):
            a_sb = wpool.tile([P, DC, D], bf16, tag="a")
            c_sb = wpool.tile([P, DC, D], bf16, tag="c")
            wr_sb = wpool.tile([P, DC], f32, tag="wr")
            nc.sync.dma_start(
                out=a_sb, in_=a_d.ap()[h].rearrange("(c p) e -> p c e", p=P)
            )
            nc.sync.dma_start(
                out=c_sb, in_=c_d.ap()[h].rearrange("(c p) e -> p c e", p=P)
            )
            nc.sync.dma_start(
                out=wr_sb, in_=wr_d.ap()[h].rearrange("(c p) -> p c", p=P)
            )

            # TA [e', s] = A^T x^T (+ wr bias)
            tat = tatp.tile([P, DC, S], bf16, tag="tat")
            for ec in range(DC):
                for sh in range(S // 512):
                    ps = ps_w.tile([P, 512], f32, tag="w")
                    for dc in range(DC):
                        nc.tensor.matmul(
                            ps,
                            a_sb[:, dc, ec * P : (ec + 1) * P],
                            xT_sb[:, dc, sh * 512 : (sh + 1) * 512],
                            start=(dc == 0),
                            stop=(dc == DC - 1),
                        )
                    nc.scalar.activation(
                        out=tat[:, ec, sh * 512 : (sh + 1) * 512],
                        in_=ps,
                        func=mybir.ActivationFunctionType.Identity,
                        bias=wr_sb[:, ec : ec + 1],
                    )

            # U [t, 0:384] = x C ; U[t, 384] = 1
            u_sb = upool.tile([P, NQT, D1], bf16, tag="u")
            nc.vector.memset(u_sb[:, :, D : D + 1], 1.0)
            for tt in range(NQT):
                ps = ps_w.tile([P, 512], f32, tag="w")
                for dc in range(DC):
                    nc.tensor.matmul(
                        ps[:, :D],
                        xT_sb[:, dc, tt * P : (tt + 1) * P],
                        c_sb[:, dc, :],
                        start=(dc == 0),
                        stop=(dc == DC - 1),
                    )
                nc.vector.tensor_copy(out=u_sb[:, tt, :D], in_=ps[:, :D])

            # attention: scoresT -> exp -> (diag-trim) per 512-wide s-group
            att_g = []
            for qg in range(NG):
                s0 = qg * 512
                ntt = qg * G4 + G4
                att = attp.tile([P, NQT, 512], bf16, tag="att", name=f"att{qg}")
                att_g.append(att)
                for tt in range(ntt):
                    jl = max(0, tt - qg * G4)  # diagonal s-offset in 128s
                    n = 512 - jl * P
                    ps = ps_qk.tile([P, 512], f32, tag="qk")
                    for ec in range(DC):
                        nc.tensor.matmul(
                            ps[:, :n],
                            xT_sb[:, ec, tt * P : (tt + 1) * P],
                            tat[:, ec, s0 + jl * P : s0 + 512],
                            start=(ec == 0),
                            stop=(ec == DC - 1),
                        )
                    nc.scalar.activation(
                        out=att[:, tt, jl * P : 512],
                        in_=ps[:, :n],
                        func=mybir.ActivationFunctionType.Exp,
                        scale=INV_SQRT_D,
                        bias=logm_sb[:, tt : tt + 1],
                    )
                    if tt >= qg * G4:
                        # zero the causally-dead triangle (s_local < t_local)
                        nc.gpsimd.affine_select(
                            out=att[:, tt, jl * P : 512],
                            in_=att[:, tt, jl * P : 512],
                            compare_op=mybir.AluOpType.is_ge,
                            fill=0.0,
                            base=0,
                            channel_multiplier=-1,
                            pattern=[[1, n]],
                        )

            # PV + normalize + accumulate, true 128-granularity causal
            for qg in range(NG):
                att = att_g[qg]
                for qi in range(G4):
                    qt = qg * G4 + qi
                    ps = ps_pv.tile([P, D1], f32, tag="pv")
                    for tt in range(qt + 1):
                        nc.tensor.matmul(
                            ps,
                            att[:, tt, qi * P : (qi + 1) * P],
                            u_sb[:, tt, :],
                            start=(tt == 0),
                            stop=(tt == qt),
                        )
                    recip = small.tile([P, 1], f32, tag="recip")
                    nc.vector.reciprocal(out=recip, in_=ps[:, D : D + 1])
                    nc.vector.scalar_tensor_tensor(
                        out=out_acc[:, qt, :],
                        in0=ps[:, :D],
                        scalar=recip,
                        in1=out_acc[:, qt, :],
                        op0=mybir.AluOpType.mult,
                        op1=mybir.AluOpType.add,
                    )

        # ---- final query-mask + store ----
        for qt in range(NQT):
            st = opool.tile([P, D], f32, tag="store")
            nc.vector.tensor_scalar_mul(
                out=st, in0=out_acc[:, qt, :], scalar1=maskq_sb[:, qt : qt + 1]
            )
            nc.sync.dma_start(out=out_d.ap()[qt * P : (qt + 1) * P, :], in_=st)

    nc.compile()
    return nc


def _in_maps(x, mask, Wq, bq, Wk, bk, Wv, bv, Wo, bo):
    import ml_dtypes

    bf16 = ml_dtypes.bfloat16
    x = np.asarray(x, np.float32)
    Wq = np.asarray(Wq, np.float32)
    Wk = np.asarray(Wk, np.float32)
    Wv = np.asarray(Wv, np.float32)
    Wo = np.asarray(Wo, np.float32).reshape(H, D, D)
    bq = np.asarray(bq, np.float32)
    bv = np.asarray(bv, np.float32)
    bo = np.asarray(bo, np.float32)

    # folded weights (exact fp32 host math)
    A = np.einsum("hde,hfe->hdf", Wq, Wk)            # scores = x A x^T
    C = np.einsum("hde,hef->hdf", Wv, Wo)            # out += attn (x C)
    wr = np.einsum("hef,hf->he", Wk, bq)             # bq rank-1 term
    bo2 = bo + np.einsum("he,hef->f", bv, Wo)        # bv term (attn sums to 1)

    m = np.asarray(mask) != 0
    logm = np.where(m, np.float32(0.0), np.float32(LOGM_DEAD)).astype(np.float32)
    maskq = m.astype(np.float32)

    shared = {
        "A": A.astype(bf16),
        "C": C.astype(bf16),
        "wr": wr,
        "bo2": np.broadcast_to(bo2[None, :], (P, D)).copy(),
    }
    return [
        {
            "xT": np.ascontiguousarray(x[b].T).astype(bf16),
            "logmT": np.ascontiguousarray(logm[b].reshape(NQT, P).T),
            "maskqT": np.ascontiguousarray(maskq[b].reshape(NQT, P).T),
            **shared,
        }
        for b in range(B)
    ]


def run(inputs, trace=False):
    """inputs: dict from setup_inputs(). Returns (out [B,S,D] f32, results)."""
    from concourse.bass_utils import run_bass_kernel_spmd

    global _BUILT
    if _BUILT is None:
        _BUILT = build()
    nc = _BUILT
    in_maps = _in_maps(**inputs)
    res = run_bass_kernel_spmd(nc, in_maps, core_ids=list(range(B)), trace=trace)
    out = np.stack([np.asarray(res.results[b]["out"], np.float32) for b in range(B)])
    return out, res


def kernel(**inputs):
    out, _ = run(inputs, trace=False)
    return out


# revision 4
# speedup vs baseline: 1.7811x; 1.0028x over previous
"""Multi-head self-attention TRN2 kernel (data-parallel over batch).

Problem: B=8, S=1024, D=384, H=8, per-head full D->D projections,
causal + key-padding mask, softmax, out_linear (H*D)->D, query-mask output.

Sharding: batch b -> NeuronCore b (8 cores, no collectives).

Host-side weight folding (exact, fp32 numpy; biases handled exactly):
  A_h = Wq_h @ Wk_h^T  -> scores_raw = x A_h x^T + 1 (x wr)^T  with
        wr = Wk_h bq_h folded as a bias into TA = x A_h (the bk rank-1
        term and bq.bk constant are per-query-row and softmax-invariant).
        The K projection disappears: the QK matmul contracts x directly
        against TA.
  C_h = Wv_h @ Wo_h    -> out = sum_h attn_h (x C_h) + bo2 with
        bo2 = bo + sum_h bv_h Wo_h (attn rows sum to 1). V projection and
        out projection collapse into one x @ C_h, and the PV matmul emits
        output-space columns directly.

Per-core dataflow (one batch element), all matmul operands bf16 (f32 psum):
  xTbf [e,s] bf16 resident in SBUF.
  For each head h:
    TA[e',s] (T-layout) = A chunkT @ xTbf (+wr bias via ACT identity)
    U[t,0:384] = xTbf chunkT @ C; U[t,384] = 1   (fused softmax denominator)
    per 512-wide s-group, causally live t-chunks, diagonal chunks trimmed
    to their live s-suffix (N = 512,384,256,128):
      scoresT[t,s] psum = xTbf chunk (lhsT) @ TA          (PE)
      att[t,s] = exp(scale*scores + logm[t])  bf16        (ACT; logm = -60
                 for mask-dead keys else 0 -- keeps dead-row colsums
                 normal-range so 1/colsum never overflows)
      diagonal chunks: zero the causally-dead triangle    (GPSIMD affine)
    per 128-wide q-tile qt (true causal granularity):
      psum[s, 0:385] = sum_{tt<=qt} att_ttT @ U_tt        (PE, one psum)
      recip = 1/psum[:,384:385]                           (DVE, per-s)
      out_acc[s,:] += psum[:, :384] * recip               (DVE STT)
  out[s,:] = out_acc * maskq[s]  -> DRAM

No DRAM bounce, no separate colsum matmuls, no mask tiles: the softmax
denominator rides along as U's 385th column, and masking is an ACT bias
plus a GPSIMD affine_select. bf16 rounding keeps rel err ~1e-2 max-normed,
inside the 2e-2 gate.
"""

import os
from contextlib import ExitStack

import numpy as np

B, S, D, H = 8, 1024, 384, 8
P = 128
DC = D // P          # 3 partition chunks of the d/e axes
NQT = S // P         # 8 q/t tiles of 128
G4 = 4               # q-tiles per s-group (s-block = 512)
NG = NQT // G4       # 2 groups
D1 = D + 1           # U carries a ones column for the softmax denominator
INV_SQRT_D = float(1.0 / np.sqrt(np.float32(D), dtype=np.float32))
LOGM_DEAD = -60.0    # exp bias for mask-dead keys: e^(score-60) stays normal

_BUILT = None


def build():
    import concourse.bass as bass
    import concourse.bacc as bacc
    import concourse.tile as tile
    import concourse.mybir as mybir

    f32 = mybir.dt.float32
    bf16 = mybir.dt.bfloat16

    nc = bacc.Bacc("TRN2", target_bir_lowering=False, debug=False)

    xT_d = nc.dram_tensor("xT", [D, S], bf16, kind="ExternalInput")
    a_d = nc.dram_tensor("A", [H, D, D], bf16, kind="ExternalInput")
    c_d = nc.dram_tensor("C", [H, D, D], bf16, kind="ExternalInput")
    wr_d = nc.dram_tensor("wr", [H, D], f32, kind="ExternalInput")
    bo2_d = nc.dram_tensor("bo2", [P, D], f32, kind="ExternalInput")
    logm_d = nc.dram_tensor("logmT", [P, NQT], f32, kind="ExternalInput")
    maskq_d = nc.dram_tensor("maskqT", [P, NQT], f32, kind="ExternalInput")
    out_d = nc.dram_tensor("out", [S, D], f32, kind="ExternalOutput")

    with tile.TileContext(nc) as tc, ExitStack() as ctx:
        consts = ctx.enter_context(tc.tile_pool(name="consts", bufs=1))
        wpool = ctx.enter_context(tc.tile_pool(name="wpool", bufs=2))
        tatp = ctx.enter_context(tc.tile_pool(name="tatp", bufs=2))
        upool = ctx.enter_context(tc.tile_pool(name="upool", bufs=2))
        attp = ctx.enter_context(tc.tile_pool(name="attp", bufs=2))
        small = ctx.enter_context(tc.tile_pool(name="small", bufs=8))
        opool = ctx.enter_context(tc.tile_pool(name="opool", bufs=2))
        ps_w = ctx.enter_context(tc.tile_pool(name="ps_w", bufs=3, space="PSUM"))
        ps_qk = ctx.enter_context(tc.tile_pool(name="ps_qk", bufs=3, space="PSUM"))
        ps_pv = ctx.enter_context(tc.tile_pool(name="ps_pv", bufs=2, space="PSUM"))

        # ---- PE warm-up under the initial DMA shadow (HAM clock ramp) ----
        warm = consts.tile([P, P], bf16, tag="warm")
        nc.vector.memset(warm, 0.0)
        wz = consts.tile([P, 512], bf16, tag="warmz")
        nc.vector.memset(wz, 0.0)
        for _ in range(20):
            ps = ps_w.tile([P, 512], f32, tag="w", name="ps_warm")
            nc.tensor.matmul(ps, warm, wz, start=True, stop=True)

        # ---- setup ----
        xT_sb = consts.tile([P, DC, S], bf16, tag="xT")
        nc.sync.dma_start(out=xT_sb, in_=xT_d.ap().rearrange("(c p) s -> p c s", p=P))

        logm_sb = consts.tile([P, NQT], f32, tag="logm")
        nc.sync.dma_start(out=logm_sb, in_=logm_d.ap())

        maskq_sb = consts.tile([P, NQT], f32, tag="maskq")
        nc.sync.dma_start(out=maskq_sb, in_=maskq_d.ap())

        bo2_sb = consts.tile([P, D], f32, tag="bo2")
        nc.sync.dma_start(out=bo2_sb, in_=bo2_d.ap())

        # out_acc is never pre-initialized: head 0's STT accumulates onto
        # bo2_sb directly (in1), later heads accumulate onto out_acc.
        out_acc = consts.tile([P, NQT, D], f32, tag="out_acc")

        # ---- per-head pipeline ----
        n_heads = int(os.environ.get("MHA_HEADS", str(H)))
        for h in range(n_heads):
            a_sb = wpool.tile([P, DC, D], bf16, tag="a")
            c_sb = wpool.tile([P, DC, D], bf16, tag="c")
            wr_sb = wpool.tile([P, DC], f32, tag="wr")
            nc.sync.dma_start(
                out=a_sb, in_=a_d.ap()[h].rearrange("(c p) e -> p c e", p=P)
            )
            nc.sync.dma_start(
                out=c_sb, in_=c_d.ap()[h].rearrange("(c p) e -> p c e", p=P)
            )
            nc.sync.dma_start(
                out=wr_sb, in_=wr_d.ap()[h].rearrange("(c p) -> p c", p=P)
            )

            # TA [e', s] = A^T x^T (+ wr bias)
            tat = tatp.tile([P, DC, S], bf16, tag="tat")
            for ec in range(DC):
                for sh in range(S // 512):
                    ps = ps_w.tile([P, 512], f32, tag="w")
                    for dc in range(DC):
                        nc.tensor.matmul(
                            ps,
                            a_sb[:, dc, ec * P : (ec + 1) * P],
                            xT_sb[:, dc, sh * 512 : (sh + 1) * 512],
                            start=(dc == 0),
                            stop=(dc == DC - 1),
                        )
                    nc.scalar.activation(
                        out=tat[:, ec, sh * 512 : (sh + 1) * 512],
                        in_=ps,
                        func=mybir.ActivationFunctionType.Identity,
                        bias=wr_sb[:, ec : ec + 1],
                    )

            # U [t, 0:384] = x C ; U[t, 384] = 1
            u_sb = upool.tile([P, NQT, D1], bf16, tag="u")
            nc.vector.memset(u_sb[:, :, D : D + 1], 1.0)
            for tt in range(NQT):
                ps = ps_w.tile([P, 512], f32, tag="w")
                for dc in range(DC):
                    nc.tensor.matmul(
                        ps[:, :D],
                        xT_sb[:, dc, tt * P : (tt + 1) * P],
                        c_sb[:, dc, :],
                        start=(dc == 0),
                        stop=(dc == DC - 1),
                    )
                nc.vector.tensor_copy(out=u_sb[:, tt, :D], in_=ps[:, :D])

            # attention: scoresT -> exp -> (diag-trim) per 512-wide s-group
            att_g = []
            for qg in range(NG):
                s0 = qg * 512
                ntt = qg * G4 + G4
                att = attp.tile([P, NQT, 512], bf16, tag="att", name=f"att{qg}")
                att_g.append(att)
                for tt in range(ntt):
                    jl = max(0, tt - qg * G4)  # diagonal s-offset in 128s
                    n = 512 - jl * P
                    ps = ps_qk.tile([P, 512], f32, tag="qk")
                    for ec in range(DC):
                        nc.tensor.matmul(
                            ps[:, :n],
                            xT_sb[:, ec, tt * P : (tt + 1) * P],
                            tat[:, ec, s0 + jl * P : s0 + 512],
                            start=(ec == 0),
                            stop=(ec == DC - 1),
                        )
                    nc.scalar.activation(
                        out=att[:, tt, jl * P : 512],
                        in_=ps[:, :n],
                        func=mybir.ActivationFunctionType.Exp,
                        scale=INV_SQRT_D,
                        bias=logm_sb[:, tt : tt + 1],
                    )
                    if tt >= qg * G4:
                        # zero the causally-dead triangle (s_local < t_local)
                        nc.gpsimd.affine_select(
                            out=att[:, tt, jl * P : 512],
                            in_=att[:, tt, jl * P : 512],
                            compare_op=mybir.AluOpType.is_ge,
                            fill=0.0,
                            base=0,
                            channel_multiplier=-1,
                            pattern=[[1, n]],
                        )

            # PV + normalize + accumulate, true 128-granularity causal
            for qg in range(NG):
                att = att_g[qg]
                for qi in range(G4):
                    qt = qg * G4 + qi
                    ps = ps_pv.tile([P, D1], f32, tag="pv")
                    for tt in range(qt + 1):
                        nc.tensor.matmul(
                            ps,
                            att[:, tt, qi * P : (qi + 1) * P],
                            u_sb[:, tt, :],
                            start=(tt == 0),
                            stop=(tt == qt),
                        )
                    recip = small.tile([P, 1], f32, tag="recip")
                    nc.vector.reciprocal(out=recip, in_=ps[:, D : D + 1])
                    nc.vector.scalar_tensor_tensor(
                        out=out_acc[:, qt, :],
                        in0=ps[:, :D],
                        scalar=recip,
                        in1=bo2_sb if h == 0 else out_acc[:, qt, :],
                        op0=mybir.AluOpType.mult,
                        op1=mybir.AluOpType.add,
                    )
                    if h == n_heads - 1:
                        # final query-mask + store, interleaved with the
                        # last head's PV tail (ACT engine is idle here)
                        st = opool.tile([P, D], f32, tag="store")
                        nc.scalar.activation(
                            out=st,
                            in_=out_acc[:, qt, :],
                            func=mybir.ActivationFunctionType.Copy,
                            scale=maskq_sb[:, qt : qt + 1],
                        )
                        nc.sync.dma_start(
                            out=out_d.ap()[qt * P : (qt + 1) * P, :], in_=st
                        )

    nc.compile()
    return nc


def _in_maps(x, mask, Wq, bq, Wk, bk, Wv, bv, Wo, bo):
    import ml_dtypes

    bf16 = ml_dtypes.bfloat16
    x = np.asarray(x, np.float32)
    Wq = np.asarray(Wq, np.float32)
    Wk = np.asarray(Wk, np.float32)
    Wv = np.asarray(Wv, np.float32)
    Wo = np.asarray(Wo, np.float32).reshape(H, D, D)
    bq = np.asarray(bq, np.float32)
    bv = np.asarray(bv, np.float32)
    bo = np.asarray(bo, np.float32)

    # folded weights (exact fp32 host math)
    A = np.einsum("hde,hfe->hdf", Wq, Wk)            # scores = x A x^T
    C = np.einsum("hde,hef->hdf", Wv, Wo)            # out += attn (x C)
    wr = np.einsum("hef,hf->he", Wk, bq)             # bq rank-1 term
    bo2 = bo + np.einsum("he,hef->f", bv, Wo)        # bv term (attn sums to 1)

    m = np.asarray(mask) != 0
    logm = np.where(m, np.float32(0.0), np.float32(LOGM_DEAD)).astype(np.float32)
    maskq = m.astype(np.float32)

    shared = {
        "A": A.astype(bf16),
        "C": C.astype(bf16),
        "wr": wr,
        "bo2": np.broadcast_to(bo2[None, :], (P, D)).copy(),
    }
    return [
        {
            "xT": np.ascontiguousarray(x[b].T).astype(bf16),
            "logmT": np.ascontiguousarray(logm[b].reshape(NQT, P).T),
            "maskqT": np.ascontiguousarray(maskq[b].reshape(NQT, P).T),
            **shared,
        }
        for b in range(B)
    ]


def run(inputs, trace=False):
    """inputs: dict from setup_inputs(). Returns (out [B,S,D] f32, results)."""
    from concourse.bass_utils import run_bass_kernel_spmd

    global _BUILT
    if _BUILT is None:
        _BUILT = build()
    nc = _BUILT
    in_maps = _in_maps(**inputs)
    res = run_bass_kernel_spmd(nc, in_maps, core_ids=list(range(B)), trace=trace)
    out = np.stack([np.asarray(res.results[b]["out"], np.float32) for b in range(B)])
    return out, res


def kernel(**inputs):
    out, _ = run(inputs, trace=False)
    return out


# revision 7
# speedup vs baseline: 1.7951x; 1.0079x over previous
"""Multi-head self-attention TRN2 kernel (data-parallel over batch).

Problem: B=8, S=1024, D=384, H=8, per-head full D->D projections,
causal + key-padding mask, softmax, out_linear (H*D)->D, query-mask output.

Sharding: batch b -> NeuronCore b (8 cores, no collectives).

Host-side weight folding (exact, fp32 numpy; biases handled exactly):
  A_h = Wq_h @ Wk_h^T  -> scores_raw = x A_h x^T + 1 (x wr)^T  with
        wr = Wk_h bq_h folded as a bias into TA = x A_h (the bk rank-1
        term and bq.bk constant are per-query-row and softmax-invariant).
        The K projection disappears: the QK matmul contracts x directly
        against TA.
  C_h = Wv_h @ Wo_h    -> out = sum_h attn_h (x C_h) + bo2 with
        bo2 = bo + sum_h bv_h Wo_h (attn rows sum to 1). V projection and
        out projection collapse into one x @ C_h, and the PV matmul emits
        output-space columns directly.

Per-core dataflow (one batch element), all matmul operands bf16 (f32 psum):
  xTbf [e,s] bf16 resident in SBUF.
  For each head h:
    TA[e',s] (T-layout) = A chunkT @ xTbf (+wr bias via ACT identity)
    U[t,0:384] = xTbf chunkT @ C; U[t,384] = 1   (fused softmax denominator)
    per 512-wide s-group, causally live t-chunks, diagonal chunks trimmed
    to their live s-suffix (N = 512,384,256,128):
      scoresT[t,s] psum = xTbf chunk (lhsT) @ TA          (PE)
      att[t,s] = exp(scale*scores + logm[t])  bf16        (ACT; logm = -60
                 for mask-dead keys else 0 -- keeps dead-row colsums
                 normal-range so 1/colsum never overflows)
      diagonal chunks: zero the causally-dead triangle    (GPSIMD affine)
    per 128-wide q-tile qt (true causal granularity):
      psum[s, 0:385] = sum_{tt<=qt} att_ttT @ U_tt        (PE, one psum)
      recip = 1/psum[:,384:385]                           (DVE, per-s)
      out_acc[s,:] += psum[:, :384] * recip               (DVE STT)
  out[s,:] = out_acc * maskq[s]  -> DRAM

No DRAM bounce, no separate colsum matmuls, no mask tiles: the softmax
denominator rides along as U's 385th column, and masking is an ACT bias
plus a GPSIMD affine_select. bf16 rounding keeps rel err ~1e-2 max-normed,
inside the 2e-2 gate.
"""

import os
from contextlib import ExitStack

import numpy as np

B, S, D, H = 8, 1024, 384, 8
P = 128
DC = D // P          # 3 partition chunks of the d/e axes
NQT = S // P         # 8 q/t tiles of 128
G4 = 4               # q-tiles per s-group (s-block = 512)
NG = NQT // G4       # 2 groups
D1 = D + 1           # U carries a ones column for the softmax denominator
INV_SQRT_D = float(1.0 / np.sqrt(np.float32(D), dtype=np.float32))
LOGM_DEAD = -60.0    # exp bias for mask-dead keys: e^(score-60) stays normal

_BUILT = None


def build():
    import concourse.bass as bass
    import concourse.bacc as bacc
    import concourse.tile as tile
    import concourse.mybir as mybir

    f32 = mybir.dt.float32
    bf16 = mybir.dt.bfloat16

    nc = bacc.Bacc("TRN2", target_bir_lowering=False, debug=False)

    xT_d = nc.dram_tensor("xT", [D, S], bf16, kind="ExternalInput")
    a_d = nc.dram_tensor("A", [H, D, D], bf16, kind="ExternalInput")
    c_d = nc.dram_tensor("C", [H, D, D], bf16, kind="ExternalInput")
    wr_d = nc.dram_tensor("wr", [H, D], f32, kind="ExternalInput")
    bo2_d = nc.dram_tensor("bo2", [P, D], f32, kind="ExternalInput")
    logm_d = nc.dram_tensor("logmT", [P, NQT], f32, kind="ExternalInput")
    maskq_d = nc.dram_tensor("maskqT", [P, NQT], f32, kind="ExternalInput")
    out_d = nc.dram_tensor("out", [S, D], f32, kind="ExternalOutput")

    with tile.TileContext(nc) as tc, ExitStack() as ctx:
        consts = ctx.enter_context(tc.tile_pool(name="consts", bufs=1))
        wpool = ctx.enter_context(tc.tile_pool(name="wpool", bufs=2))
        tatp = ctx.enter_context(tc.tile_pool(name="tatp", bufs=2))
        upool = ctx.enter_context(tc.tile_pool(name="upool", bufs=2))
        attp = ctx.enter_context(tc.tile_pool(name="attp", bufs=2))
        small = ctx.enter_context(tc.tile_pool(name="small", bufs=8))
        opool = ctx.enter_context(tc.tile_pool(name="opool", bufs=4))
        ps_w = ctx.enter_context(tc.tile_pool(name="ps_w", bufs=3, space="PSUM"))
        ps_qk = ctx.enter_context(tc.tile_pool(name="ps_qk", bufs=3, space="PSUM"))
        ps_pv = ctx.enter_context(tc.tile_pool(name="ps_pv", bufs=2, space="PSUM"))

        # ---- setup: spread the startup loads over BOTH hardware DMA
        # queues (Sync + Scalar are the two hwdge engines) so the first
        # head's TA matmuls can start as early as possible.
        xT_sb = consts.tile([P, DC, S], bf16, tag="xT")
        for sh in range(2):
            nc.sync.dma_start(
                out=xT_sb[:, :, sh * 512 : (sh + 1) * 512],
                in_=xT_d.ap()[:, sh * 512 : (sh + 1) * 512].rearrange(
                    "(c p) s -> p c s", p=P
                ),
            )

        logm_sb = consts.tile([P, NQT], f32, tag="logm")
        nc.scalar.dma_start(out=logm_sb, in_=logm_d.ap())

        maskq_sb = consts.tile([P, NQT], f32, tag="maskq")
        nc.scalar.dma_start(out=maskq_sb, in_=maskq_d.ap())

        bo2_sb = consts.tile([P, D], f32, tag="bo2")
        nc.scalar.dma_start(out=bo2_sb, in_=bo2_d.ap())

        # ---- PE warm-up under the initial DMA shadow (HAM clock ramp) ----
        warm = consts.tile([P, P], bf16, tag="warm")
        nc.vector.memset(warm, 0.0)
        wz = consts.tile([P, 512], bf16, tag="warmz")
        nc.vector.memset(wz, 0.0)
        for _ in range(12):
            ps = ps_w.tile([P, 512], f32, tag="w", name="ps_warm")
            nc.tensor.matmul(ps, warm, wz, start=True, stop=True)

        # out_acc is never pre-initialized: head 0's STT accumulates onto
        # bo2_sb directly (in1), later heads accumulate onto out_acc.
        out_acc = consts.tile([P, NQT, D], f32, tag="out_acc")

        # ---- per-head pipeline ----
        n_heads = int(os.environ.get("MHA_HEADS", str(H)))
        for h in range(n_heads):
            a_sb = wpool.tile([P, DC, D], bf16, tag="a")
            c_sb = wpool.tile([P, DC, D], bf16, tag="c")
            wr_sb = wpool.tile([P, DC], f32, tag="wr")
            nc.scalar.dma_start(
                out=a_sb, in_=a_d.ap()[h].rearrange("(c p) e -> p c e", p=P)
            )
            nc.sync.dma_start(
                out=c_sb, in_=c_d.ap()[h].rearrange("(c p) e -> p c e", p=P)
            )
            nc.scalar.dma_start(
                out=wr_sb, in_=wr_d.ap()[h].rearrange("(c p) -> p c", p=P)
            )

            # TA [e', s] = A^T x^T (+ wr bias); sh-major so the first
            # half of xT (own DMA) unblocks the first 9 matmuls
            tat = tatp.tile([P, DC, S], bf16, tag="tat")
            for sh in range(S // 512):
                for ec in range(DC):
                    ps = ps_w.tile([P, 512], f32, tag="w")
                    for dc in range(DC):
                        nc.tensor.matmul(
                            ps,
                            a_sb[:, dc, ec * P : (ec + 1) * P],
                            xT_sb[:, dc, sh * 512 : (sh + 1) * 512],
                            start=(dc == 0),
                            stop=(dc == DC - 1),
                        )
                    nc.scalar.activation(
                        out=tat[:, ec, sh * 512 : (sh + 1) * 512],
                        in_=ps,
                        func=mybir.ActivationFunctionType.Identity,
                        bias=wr_sb[:, ec : ec + 1],
                    )

            # U [t, 0:384] = x C ; U[t, 384] = 1
            u_sb = upool.tile([P, NQT, D1], bf16, tag="u")
            nc.vector.memset(u_sb[:, :, D : D + 1], 1.0)
            for tt in range(NQT):
                ps = ps_w.tile([P, 512], f32, tag="w")
                for dc in range(DC):
                    nc.tensor.matmul(
                        ps[:, :D],
                        xT_sb[:, dc, tt * P : (tt + 1) * P],
                        c_sb[:, dc, :],
                        start=(dc == 0),
                        stop=(dc == DC - 1),
                    )
                nc.vector.tensor_copy(out=u_sb[:, tt, :D], in_=ps[:, :D])

            # attention: scoresT -> exp -> (diag-trim) per 512-wide s-group
            att_g = []
            for qg in range(NG):
                s0 = qg * 512
                ntt = qg * G4 + G4
                att = attp.tile([P, NQT, 512], bf16, tag="att", name=f"att{qg}")
                att_g.append(att)
                for tt in range(ntt):
                    jl = max(0, tt - qg * G4)  # diagonal s-offset in 128s
                    n = 512 - jl * P
                    ps = ps_qk.tile([P, 512], f32, tag="qk")
                    for ec in range(DC):
                        nc.tensor.matmul(
                            ps[:, :n],
                            xT_sb[:, ec, tt * P : (tt + 1) * P],
                            tat[:, ec, s0 + jl * P : s0 + 512],
                            start=(ec == 0),
                            stop=(ec == DC - 1),
                        )
                    nc.scalar.activation(
                        out=att[:, tt, jl * P : 512],
                        in_=ps[:, :n],
                        func=mybir.ActivationFunctionType.Exp,
                        scale=INV_SQRT_D,
                        bias=logm_sb[:, tt : tt + 1],
                    )
                    if tt >= qg * G4:
                        # zero the causally-dead triangle (s_local < t_local)
                        nc.gpsimd.affine_select(
                            out=att[:, tt, jl * P : 512],
                            in_=att[:, tt, jl * P : 512],
                            compare_op=mybir.AluOpType.is_ge,
                            fill=0.0,
                            base=0,
                            channel_multiplier=-1,
                            pattern=[[1, n]],
                        )

            # PV + normalize + accumulate, true 128-granularity causal
            for qg in range(NG):
                att = att_g[qg]
                for qi in range(G4):
                    qt = qg * G4 + qi
                    ps = ps_pv.tile([P, D1], f32, tag="pv")
                    for tt in range(qt + 1):
                        nc.tensor.matmul(
                            ps,
                            att[:, tt, qi * P : (qi + 1) * P],
                            u_sb[:, tt, :],
                            start=(tt == 0),
                            stop=(tt == qt),
                        )
                    recip = small.tile([P, 1], f32, tag="recip")
                    nc.vector.reciprocal(out=recip, in_=ps[:, D : D + 1])
                    nc.vector.scalar_tensor_tensor(
                        out=out_acc[:, qt, :],
                        in0=ps[:, :D],
                        scalar=recip,
                        in1=bo2_sb if h == 0 else out_acc[:, qt, :],
                        op0=mybir.AluOpType.mult,
                        op1=mybir.AluOpType.add,
                    )
                    if h == n_heads - 1:
                        # final query-mask + store, interleaved with the
                        # last head's PV tail (ACT engine is idle here)
                        st = opool.tile([P, D], f32, tag="store")
                        nc.scalar.activation(
                            out=st,
                            in_=out_acc[:, qt, :],
                            func=mybir.ActivationFunctionType.Copy,
                            scale=maskq_sb[:, qt : qt + 1],
                        )
                        nc.sync.dma_start(
                            out=out_d.ap()[qt * P : (qt + 1) * P, :], in_=st
                        )

    nc.compile()
    return nc


def _in_maps(x, mask, Wq, bq, Wk, bk, Wv, bv, Wo, bo):
    import ml_dtypes

    bf16 = ml_dtypes.bfloat16
    x = np.asarray(x, np.float32)
    Wq = np.asarray(Wq, np.float32)
    Wk = np.asarray(Wk, np.float32)
    Wv = np.asarray(Wv, np.float32)
    Wo = np.asarray(Wo, np.float32).reshape(H, D, D)
    bq = np.asarray(bq, np.float32)
    bv = np.asarray(bv, np.float32)
    bo = np.asarray(bo, np.float32)

    # folded weights (exact fp32 host math)
    A = np.einsum("hde,hfe->hdf", Wq, Wk)            # scores = x A x^T
    C = np.einsum("hde,hef->hdf", Wv, Wo)            # out += attn (x C)
    wr = np.einsum("hef,hf->he", Wk, bq)             # bq rank-1 term
    bo2 = bo + np.einsum("he,hef->f", bv, Wo)        # bv term (attn sums to 1)

    m = np.asarray(mask) != 0
    logm = np.where(m, np.float32(0.0), np.float32(LOGM_DEAD)).astype(np.float32)
    maskq = m.astype(np.float32)

    shared = {
        "A": A.astype(bf16),
        "C": C.astype(bf16),
        "wr": wr,
        "bo2": np.broadcast_to(bo2[None, :], (P, D)).copy(),
    }
    return [
        {
            "xT": np.ascontiguousarray(x[b].T).astype(bf16),
            "logmT": np.ascontiguousarray(logm[b].reshape(NQT, P).T),
            "maskqT": np.ascontiguousarray(maskq[b].reshape(NQT, P).T),
            **shared,
        }
        for b in range(B)
    ]


def run(inputs, trace=False):
    """inputs: dict from setup_inputs(). Returns (out [B,S,D] f32, results)."""
    from concourse.bass_utils import run_bass_kernel_spmd

    global _BUILT
    if _BUILT is None:
        _BUILT = build()
    nc = _BUILT
    in_maps = _in_maps(**inputs)
    res = run_bass_kernel_spmd(nc, in_maps, core_ids=list(range(B)), trace=trace)
    out = np.stack([np.asarray(res.results[b]["out"], np.float32) for b in range(B)])
    return out, res


def kernel(**inputs):
    out, _ = run(inputs, trace=False)
    return out
